# revision 18
# baseline (speedup 1.0000x reference)
# Trainium2 Bass kernel for nn_Decoder (RNN decoder):
#   xp = x @ W_ih^T + b_ih + b_hh            (GEMM1, bf16)
#   h_t = tanh(xp_t + h_{t-1} @ W_hh^T)      (512-step recurrence, bf16)
#   y  = hs @ W_ff^T + b_ff                  (GEMM2, bf16)
#
# Sharding: data-parallel over batch, 8 batch rows per core, weights replicated.
# Layouts are hidden-on-partitions so the sequential recurrence needs no
# transposes: h is stored [hid(4x128 part-tiles), batch(8)].
#
# Per-step critical chain (measured on HW):
#   tanhY done -> sem(~52) -> Y-group matmuls (26.5ns/MM pair, LDW-rate-bound)
#   -> drain(~167) -> sem(~116) -> tanhY (ACT ~273)
# so the schedule minimizes the gating group's matmul count with a 2+2
# stagger: hid tiles {2,3} (Y, gating) and {0,1} (X, slack side). Each z half
# lives in ONE psum bank [128, 2, 8] so a single identity-matmul injects xp
# for the half and a single ACT tanh drains it.
#
# GEMM1 (after its first chunk) and GEMM2 are chunked small and emitted one
# piece per step into the tanh shadow; big lumps would stall the gating burst.
# GEMM2 chunks are gated on which h history slots exist.

import numpy as np
import ml_dtypes

B, S, I, H, O = 64, 512, 256, 512, 256
NCORES = 8
BL = B // NCORES  # 8 batch rows per core
P = 128
KI, KH, KO = I // P, H // P, O // P  # 2, 4, 2
CH = 512                             # free-dim chunk for GEMM1
CH2 = 64                             # free-dim chunk for interleaved GEMM2
_builder_cache = {}


def build_nc(seq_len=S):
    """Build the (single-core SPMD) Bass program for sequence length seq_len."""
    import concourse.bass as bass
    import concourse.mybir as mybir
    import concourse.tile as tile
    from concourse import bacc
    from concourse.tile import add_dep_helper

    f32 = mybir.dt.float32
    bf16 = mybir.dt.bfloat16
    AF = mybir.ActivationFunctionType

    s = seq_len
    assert s % 2 == 0
    F = s * BL               # free length of (t, b) axis
    nch = max(1, F // CH)    # chunks for GEMM1
    ch = F // nch
    F2 = F // 2              # per-parity free length for GEMM2
    nch2 = max(1, F2 // CH2)

    nc = bacc.Bacc("TRN2")

    xt = nc.dram_tensor("xt", [I, F], bf16, kind="ExternalInput")      # x^T  (in, t*BL+b)
    h0t = nc.dram_tensor("h0t", [H, BL], bf16, kind="ExternalInput")   # h0^T (hid, b)
    wih = nc.dram_tensor("wih", [I, H], bf16, kind="ExternalInput")    # W_ih^T
    whh = nc.dram_tensor("whh", [H, H], bf16, kind="ExternalInput")    # W_hh^T
    wff = nc.dram_tensor("wff", [H, O], bf16, kind="ExternalInput")    # W_ff^T
    bcb = nc.dram_tensor("bcb", [P, KH], f32, kind="ExternalInput")    # b_ih+b_hh, [128, 4]
    bfb = nc.dram_tensor("bfb", [P, KO], f32, kind="ExternalInput")    # b_ff, [128, 2]
    eye = nc.dram_tensor("eye", [P, P], bf16, kind="ExternalInput")
    # y[ot, p, par, q*BL + b]:  par=0 -> t = 2q+1, par=1 -> t = 2q
    y = nc.dram_tensor("y", [KO, P, 2, F2], f32, kind="ExternalOutput")

    with tile.TileContext(nc) as tc:
        with (
            tc.tile_pool(name="const", bufs=1) as cp,
            tc.tile_pool(name="big", bufs=1) as bp,
        ):
            wih_sb = cp.tile([P, KI, H], bf16)
            whh_sb = cp.tile([P, KH, H], bf16)
            wff_sb = cp.tile([P, KH, O], bf16)
            bcb_sb = cp.tile([P, KH], f32)
            bfb_sb = cp.tile([P, KO], f32)
            eye_sb = cp.tile([P, P], bf16)

            xt_sb = bp.tile([P, KI, F], bf16)
            xp_sb = bp.tile([P, KH, F], bf16)
            # h_i (i = t+1, 0..s) lives in parity tensor (i % 2) at col-slot
            # (i // 2) * BL; the X tensor holds hid-tiles {0,1}, Y holds {2,3}.
            n0 = (s // 2 + 1) * BL
            n1 = (s // 2) * BL
            hs0X = bp.tile([P, 2, n0], bf16)
            hs0Y = bp.tile([P, 2, n0], bf16)
            hs1X = bp.tile([P, 2, n1], bf16)
            hs1Y = bp.tile([P, 2, n1], bf16)
            hX = [hs0X, hs1X]
            hY = [hs0Y, hs1Y]
            out_sb = bp.tile([P, KO, 2, F2], f32)

            # ---- input loads (ordered so the recurrence prerequisites land
            # first; xt chunks 1.. and the GEMM2 weights trail) ----
            xt_r = xt[:].rearrange("(k p) f -> p k f", p=P)
            whh_r = whh[:].rearrange("(k p) h -> p k h", p=P)
            # spread the prologue loads over four engine DMA queues (the
            # recurrence start is gated on whh+wih+xt0 transfer time; a
            # single queue runs ~73GB/s)
            # whh arrives as quadrants in the burst's consumption order:
            # YxkY reads [2:4, 256:], XxkY [2:4, :256], YxkX [0:2, 256:],
            # XxkX [0:2, :256] -- so the first burst can start before the
            # whole 512KB has landed.
            HH = H // 2
            nc.scalar.dma_start(wih_sb[:], wih[:].rearrange("(k p) h -> p k h", p=P))
            nc.scalar.dma_start(whh_sb[:, 0, HH:H], whh_r[:, 0, HH:H])
            nc.gpsimd.dma_start(whh_sb[:, 1:2, :], whh_r[:, 1:2, :])
            nc.sync.dma_start(eye_sb[:], eye[:])
            nc.sync.dma_start(xt_sb[:, :, 0:64], xt_r[:, :, 0:64])
            nc.sync.dma_start(bcb_sb[:], bcb[:])
            h0r = h0t[:].rearrange("(k p) b -> p k b", p=P)
            nc.sync.dma_start(hs0X[:, :, 0:BL], h0r[:, 0:2, :])
            nc.sync.dma_start(hs0Y[:, :, 0:BL], h0r[:, 2:KH, :])
            nc.sync.dma_start(whh_sb[:, 2:KH, HH:H], whh_r[:, 2:KH, HH:H])
            nc.sync.dma_start(whh_sb[:, 2:KH, 0:HH], whh_r[:, 2:KH, 0:HH])
            nc.sync.dma_start(whh_sb[:, 0, 0:HH], whh_r[:, 0, 0:HH])
            nc.sync.dma_start(bfb_sb[:], bfb[:])
            nc.sync.dma_start(xt_sb[:, :, 64:ch], xt_r[:, :, 64:ch])
            nc.gpsimd.dma_start(wff_sb[:], wff[:].rearrange("(k p) o -> p k o", p=P))

            for j in range(1, nch):
                sl2 = slice(j * ch, (j + 1) * ch)
                nc.sync.dma_start(xt_sb[:, :, sl2], xt_r[:, :, sl2])

            y_r = y[:].rearrange("o p q f -> p o q f")

            with (
                tc.tile_pool(name="g1ps", bufs=2, space=bass.MemorySpace.PSUM) as g1p,
                tc.tile_pool(name="zXps", bufs=2, space=bass.MemorySpace.PSUM) as zXp,
                tc.tile_pool(name="zYps", bufs=2, space=bass.MemorySpace.PSUM) as zYp,
                tc.tile_pool(name="g2ps", bufs=2, space=bass.MemorySpace.PSUM) as g2p,
            ):
                # GEMM1 work units: (j, m) = KI matmuls (N=ch) + drain into
                # xp. Chunk 0 runs up front (the recurrence needs it at step
                # 0); the rest interleave into the recurrence's tanh shadow.
                G1W = 64
                ng1 = F // G1W

                def g1_emit(j, m, after):
                    sl = slice(j * G1W, (j + 1) * G1W)
                    ps = g1p.tile([P, G1W], f32, tag="g1ps", name="g1t")
                    for k in range(KI):
                        e = nc.tensor.matmul(
                            ps[:],
                            wih_sb[:, k, m * P : (m + 1) * P],
                            xt_sb[:, k, sl],
                            start=(k == 0),
                            stop=(k == KI - 1),
                        )
                        if after is not None:
                            add_dep_helper(e.ins, after.ins, sync=False)
                        after = e
                    nc.vector.tensor_scalar_add(
                        xp_sb[:, m, sl], ps[:], bcb_sb[:, m : m + 1]
                    )
                    return after

                prev = None
                for m in range(KH):
                    prev = g1_emit(0, m, prev)
                g1_jobs = [(j, m) for j in range(1, ng1) for m in range(KH)]
                g1_i = 0

                # GEMM2 work units: (par, lo, hi, ot, gate) = KH matmuls
                # (N=hi-lo) + 1 DVE drain (+ the piece's output DMA on the
                # last ot). A piece reads h slots that exist after step
                # `gate`. The final parity-0 chunk is split 56+8 so only the
                # 8 columns depending on step 511 trail the loop.
                g2_jobs = []
                for c in range(nch2):
                    for par in range(2):
                        lo, hi = c * CH2, (c + 1) * CH2
                        gate = 16 * c + 16 - par
                        if par == 0 and c == nch2 - 1:
                            for ot in range(KO):
                                g2_jobs.append((0, lo, hi - BL, ot, gate - 2))
                            for ot in range(KO):
                                g2_jobs.append((0, hi - BL, hi, ot, gate))
                        else:
                            for ot in range(KO):
                                g2_jobs.append((par, lo, hi, ot, gate))
                g2_i = 0

                def g2_emit(job, after):
                    par, lo, hi, ot, _ = job
                    base = BL if par == 0 else 0
                    sl = slice(lo, hi)
                    hsl = slice(base + lo, base + hi)
                    ps = g2p.tile([P, hi - lo], f32, tag="g2ps",
                                  padded_shape=[P, CH2])
                    for k in range(KH):
                        rhs = (
                            hX[par][:, k, hsl] if k < 2 else hY[par][:, k - 2, hsl]
                        )
                        e = nc.tensor.matmul(
                            ps[:],
                            wff_sb[:, k, ot * P : (ot + 1) * P],
                            rhs,
                            start=(k == 0),
                            stop=(k == KH - 1),
                        )
                        if after is not None:
                            add_dep_helper(e.ins, after.ins, sync=False)
                        after = e
                    nc.vector.tensor_scalar_add(
                        out_sb[:, ot, par, sl], ps[:], bfb_sb[:, ot : ot + 1]
                    )
                    if ot == KO - 1:
                        eng = nc.gpsimd if (lo // CH2 + par) % 2 and lo // CH2 < 29 else nc.sync
                        eng.dma_start(y_r[:, :, par, sl], out_sb[:, :, par, sl])
                    return after

                # ---- the recurrence ----
                def kmm(zX, zY, m, k, t):
                    rX, rY = hX[t % 2], hY[t % 2]
                    rof = (t // 2) * BL
                    zt = zX[:, m, :] if m < 2 else zY[:, m - 2, :]
                    rhs = (
                        rX[:, k, rof : rof + BL]
                        if k < 2
                        else rY[:, k - 2, rof : rof + BL]
                    )
                    return nc.tensor.matmul(
                        zt,
                        whh_sb[:, k, m * P : (m + 1) * P],
                        rhs,
                        start=False,
                        stop=(m < 2 and k == 1) or (m >= 2 and k == 1),
                    )

                def imm(zt, mlo, t):
                    # single identity-matmul per psum half injects xp_t
                    return nc.tensor.matmul(
                        zt[:],
                        eye_sb[:],
                        xp_sb[:, mlo : mlo + 2, t * BL : (t + 1) * BL],
                        start=True,
                        stop=False,
                    )

                def chain(e):
                    nonlocal prev
                    add_dep_helper(e.ins, prev.ins, sync=False)
                    prev = e

                zX = zXp.tile([P, 2, BL], f32, tag="zx")
                zY = zYp.tile([P, 2, BL], f32, tag="zy")
                chain(imm(zY, 2, 0))
                chain(imm(zX, 0, 0))
                for t in range(s):
                    wX, wY = hX[(t + 1) % 2], hY[(t + 1) % 2]
                    wof = ((t + 1) // 2) * BL

                    # Burst order so the X half completes early enough for
                    # tanhX to PIPELINE behind tanhY on ACT (issue-to-issue
                    # ~179ns) instead of serializing (+273): Y*kY, X*kY,
                    # Y*kX (gating, lands past the now-earlier tanhX gate
                    # with no stall), tanhY, X*kX, tanhX.
                    # Y group (gating): k in Y tiles first, k in X tiles last
                    for m in (2, 3):
                        for k in (3, 2):
                            chain(kmm(zX, zY, m, k, t))
                    # next step's zY bank + xp inject fills the X-wait gap
                    if t + 1 < s:
                        zY2 = zYp.tile([P, 2, BL], f32, tag="zy")
                        chain(imm(zY2, 2, t + 1))
                    for m in (2, 3):
                        for k in (0, 1):
                            chain(kmm(zX, zY, m, k, t))
                    nc.scalar.activation(
                        wY[:, :, wof : wof + BL], zY[:], AF.Tanh
                    )
                    # X group
                    for m in (0, 1):
                        for k in (3, 2, 0, 1):
                            chain(kmm(zX, zY, m, k, t))
                    nc.scalar.activation(
                        wX[:, :, wof : wof + BL], zX[:], AF.Tanh
                    )
                    if t + 1 < s:
                        zX2 = zXp.tile([P, 2, BL], f32, tag="zx")
                        chain(imm(zX2, 0, t + 1))
                        zX, zY = zX2, zY2
                    # shadow jobs per step: GEMM1 matmuls first, then GEMM2
                    if g1_i < len(g1_jobs):
                        j, m = g1_jobs[g1_i]
                        prev = g1_emit(j, m, prev)
                        g1_i += 1
                    else:
                        for _ in range(2 if t >= 496 else 1):
                            if g2_i < len(g2_jobs) and g2_jobs[g2_i][4] <= t:
                                prev = g2_emit(g2_jobs[g2_i], prev)
                                g2_i += 1
                # tail of GEMM2 (last chunks need the final steps)
                while g2_i < len(g2_jobs):
                    prev = g2_emit(g2_jobs[g2_i], prev)
                    g2_i += 1

    return nc


def make_in_maps(x, h0, W_ih, W_hh, b_ih, b_hh, W_ff, b_ff, seq_len=S):
    """Host-side sharding + layout prep: per-core input dicts."""
    bf = ml_dtypes.bfloat16
    x = np.asarray(x, np.float32)
    h0 = np.asarray(h0, np.float32)
    wih = np.ascontiguousarray(np.asarray(W_ih, np.float32).T).astype(bf)   # [I, H]
    whh = np.ascontiguousarray(np.asarray(W_hh, np.float32).T).astype(bf)   # [H, H]
    wff = np.ascontiguousarray(np.asarray(W_ff, np.float32).T).astype(bf)   # [H, O]
    bc = np.asarray(b_ih, np.float32) + np.asarray(b_hh, np.float32)
    bcb = np.ascontiguousarray(bc.reshape(KH, P).T)             # [128, KH]
    bfb = np.ascontiguousarray(np.asarray(b_ff, np.float32).reshape(KO, P).T)
    eye = np.eye(P, dtype=np.float32).astype(bf)

    in_maps = []
    for c in range(NCORES):
        xs = x[c * BL : (c + 1) * BL, :seq_len]                 # [BL, s, I]
        xt = np.ascontiguousarray(xs.transpose(2, 1, 0)).reshape(I, seq_len * BL)
        h0t = np.ascontiguousarray(h0[c * BL : (c + 1) * BL].T)  # [H, BL]
        in_maps.append(
            {
                "xt": xt.astype(bf),
                "h0t": h0t.astype(bf),
                "wih": wih,
                "whh": whh,
                "wff": wff,
                "bcb": bcb,
                "bfb": bfb,
                "eye": eye,
            }
        )
    return in_maps


def assemble_output(results, seq_len=S):
    """Per-core y [KO, 128, 2, (s/2)*BL] -> full [B, s, O]."""
    s = seq_len
    outs = []
    for r in results:
        yc = np.asarray(r["y"]).reshape(O, 2, s // 2, BL)
        full = np.empty((O, s, BL), np.float32)
        full[:, 1::2, :] = yc[:, 0]   # par=0: t = 2q+1
        full[:, 0::2, :] = yc[:, 1]   # par=1: t = 2q
        outs.append(full.transpose(2, 1, 0))
    return np.ascontiguousarray(np.concatenate(outs, axis=0))


def _get_finalized_nc(seq_len=S):
    key = ("nc", seq_len)
    if key not in _builder_cache:
        nc = build_nc(seq_len)
        nc.finalize()
        _builder_cache[key] = nc
    return _builder_cache[key]


def run_on_cores(inputs, seq_len=S, **kwargs):
    from concourse.bass_utils import run_bass_kernel_spmd

    nc = _get_finalized_nc(seq_len)
    in_maps = make_in_maps(**inputs, seq_len=seq_len)
    res = run_bass_kernel_spmd(nc, in_maps, core_ids=list(range(NCORES)), **kwargs)
    return res


def kernel(**inputs) -> np.ndarray:
    res = run_on_cores(inputs)
    return assemble_output(res.results)


# revision 19
# speedup vs baseline: 1.0018x; 1.0018x over previous
# Trainium2 Bass kernel for nn_Decoder (RNN decoder):
#   xp = x @ W_ih^T + b_ih + b_hh            (GEMM1, bf16)
#   h_t = tanh(xp_t + h_{t-1} @ W_hh^T)      (512-step recurrence, bf16)
#   y  = hs @ W_ff^T + b_ff                  (GEMM2, bf16)
#
# Sharding: data-parallel over batch, 8 batch rows per core, weights replicated.
# Layouts are hidden-on-partitions so the sequential recurrence needs no
# transposes: h is stored [hid(4x128 part-tiles), batch(8)].
#
# Per-step critical cycle (measured on HW, 904.6ns steady):
#   tanhY (ACT 273) -> tanhX (ACT, serialized +273) -> sem(51) ->
#   4 Y*kX matmuls (26.5ns/pair dispatch floor) -> drain(167) -> sem(60)
# The 2+2 stagger (hid tiles {2,3} = Y gating, {0,1} = X) minimizes the
# matmul count on that cycle; all other orderings re-converge to ~904
# (the framework's +4-op semaphore lookahead on ACT waits cancels any
# ACT-pipelining gain). Each z half lives in ONE psum bank [128, 2, 8] so
# a single identity-matmul injects xp and a single ACT tanh drains it.
#
# GEMM1 (after its first chunk) and GEMM2 are chunked small and emitted one
# piece per step into the tanh shadow; big lumps would stall the gating burst.
# GEMM2 chunks are gated on which h history slots exist.

import numpy as np
import ml_dtypes

B, S, I, H, O = 64, 512, 256, 512, 256
NCORES = 8
BL = B // NCORES  # 8 batch rows per core
P = 128
KI, KH, KO = I // P, H // P, O // P  # 2, 4, 2
CH = 512                             # free-dim chunk for GEMM1
CH2 = 64                             # free-dim chunk for interleaved GEMM2
_builder_cache = {}


def build_nc(seq_len=S):
    """Build the (single-core SPMD) Bass program for sequence length seq_len."""
    import concourse.bass as bass
    import concourse.mybir as mybir
    import concourse.tile as tile
    from concourse import bacc
    from concourse.tile import add_dep_helper

    f32 = mybir.dt.float32
    bf16 = mybir.dt.bfloat16
    AF = mybir.ActivationFunctionType

    s = seq_len
    assert s % 2 == 0
    F = s * BL               # free length of (t, b) axis
    nch = max(1, F // CH)    # chunks for GEMM1
    ch = F // nch
    F2 = F // 2              # per-parity free length for GEMM2
    nch2 = max(1, F2 // CH2)

    nc = bacc.Bacc("TRN2")

    xt = nc.dram_tensor("xt", [I, F], bf16, kind="ExternalInput")      # x^T  (in, t*BL+b)
    h0t = nc.dram_tensor("h0t", [H, BL], bf16, kind="ExternalInput")   # h0^T (hid, b)
    wih = nc.dram_tensor("wih", [I, H], bf16, kind="ExternalInput")    # W_ih^T
    whh = nc.dram_tensor("whh", [H, H], bf16, kind="ExternalInput")    # W_hh^T
    wff = nc.dram_tensor("wff", [H, O], bf16, kind="ExternalInput")    # W_ff^T
    bcb = nc.dram_tensor("bcb", [P, KH], f32, kind="ExternalInput")    # b_ih+b_hh, [128, 4]
    bfb = nc.dram_tensor("bfb", [P, KO], f32, kind="ExternalInput")    # b_ff, [128, 2]
    eye = nc.dram_tensor("eye", [P, P], bf16, kind="ExternalInput")
    # y[ot, p, par, q*BL + b]:  par=0 -> t = 2q+1, par=1 -> t = 2q
    y = nc.dram_tensor("y", [KO, P, 2, F2], f32, kind="ExternalOutput")

    with tile.TileContext(nc) as tc:
        with (
            tc.tile_pool(name="const", bufs=1) as cp,
            tc.tile_pool(name="big", bufs=1) as bp,
        ):
            wih_sb = cp.tile([P, KI, H], bf16)
            whh_sb = cp.tile([P, KH, H], bf16)
            wff_sb = cp.tile([P, KH, O], bf16)
            bcb_sb = cp.tile([P, KH], f32)
            bfb_sb = cp.tile([P, KO], f32)
            eye_sb = cp.tile([P, P], bf16)

            xt_sb = bp.tile([P, KI, F], bf16)
            xp_sb = bp.tile([P, KH, F], bf16)
            # h_i (i = t+1, 0..s) lives in parity tensor (i % 2) at col-slot
            # (i // 2) * BL; the X tensor holds hid-tiles {0,1}, Y holds {2,3}.
            n0 = (s // 2 + 1) * BL
            n1 = (s // 2) * BL
            hs0X = bp.tile([P, 2, n0], bf16)
            hs0Y = bp.tile([P, 2, n0], bf16)
            hs1X = bp.tile([P, 2, n1], bf16)
            hs1Y = bp.tile([P, 2, n1], bf16)
            hX = [hs0X, hs1X]
            hY = [hs0Y, hs1Y]
            out_sb = bp.tile([P, KO, 2, F2], f32)

            # ---- input loads (ordered so the recurrence prerequisites land
            # first; xt chunks 1.. and the GEMM2 weights trail) ----
            xt_r = xt[:].rearrange("(k p) f -> p k f", p=P)
            whh_r = whh[:].rearrange("(k p) h -> p k h", p=P)
            # spread the prologue loads over four engine DMA queues (the
            # recurrence start is gated on whh+wih+xt0 transfer time; a
            # single queue runs ~73GB/s)
            # whh arrives as quadrants in the burst's consumption order:
            # YxkY reads [2:4, 256:], XxkY [2:4, :256], YxkX [0:2, 256:],
            # XxkX [0:2, :256] -- so the first burst can start before the
            # whole 512KB has landed.
            HH = H // 2
            nc.scalar.dma_start(wih_sb[:], wih[:].rearrange("(k p) h -> p k h", p=P))
            nc.scalar.dma_start(whh_sb[:, 0, HH:H], whh_r[:, 0, HH:H])
            nc.gpsimd.dma_start(whh_sb[:, 1:2, :], whh_r[:, 1:2, :])
            nc.sync.dma_start(eye_sb[:], eye[:])
            nc.sync.dma_start(xt_sb[:, :, 0:64], xt_r[:, :, 0:64])
            nc.sync.dma_start(bcb_sb[:], bcb[:])
            h0r = h0t[:].rearrange("(k p) b -> p k b", p=P)
            nc.sync.dma_start(hs0X[:, :, 0:BL], h0r[:, 0:2, :])
            nc.sync.dma_start(hs0Y[:, :, 0:BL], h0r[:, 2:KH, :])
            nc.sync.dma_start(whh_sb[:, 2:KH, HH:H], whh_r[:, 2:KH, HH:H])
            nc.sync.dma_start(whh_sb[:, 2:KH, 0:HH], whh_r[:, 2:KH, 0:HH])
            nc.sync.dma_start(whh_sb[:, 0, 0:HH], whh_r[:, 0, 0:HH])
            nc.sync.dma_start(bfb_sb[:], bfb[:])
            nc.sync.dma_start(xt_sb[:, :, 64:ch], xt_r[:, :, 64:ch])
            nc.gpsimd.dma_start(wff_sb[:], wff[:].rearrange("(k p) o -> p k o", p=P))

            for j in range(1, nch):
                sl2 = slice(j * ch, (j + 1) * ch)
                nc.sync.dma_start(xt_sb[:, :, sl2], xt_r[:, :, sl2])

            y_r = y[:].rearrange("o p q f -> p o q f")

            with (
                tc.tile_pool(name="g1ps", bufs=2, space=bass.MemorySpace.PSUM) as g1p,
                tc.tile_pool(name="zXps", bufs=2, space=bass.MemorySpace.PSUM) as zXp,
                tc.tile_pool(name="zYps", bufs=2, space=bass.MemorySpace.PSUM) as zYp,
                tc.tile_pool(name="g2ps", bufs=2, space=bass.MemorySpace.PSUM) as g2p,
            ):
                # GEMM1 work units: (j, m) = KI matmuls (N=ch) + drain into
                # xp. Chunk 0 runs up front (the recurrence needs it at step
                # 0); the rest interleave into the recurrence's tanh shadow.
                G1W = 64
                ng1 = F // G1W

                def g1_emit(j, m, after):
                    sl = slice(j * G1W, (j + 1) * G1W)
                    ps = g1p.tile([P, G1W], f32, tag="g1ps", name="g1t")
                    for k in range(KI):
                        e = nc.tensor.matmul(
                            ps[:],
                            wih_sb[:, k, m * P : (m + 1) * P],
                            xt_sb[:, k, sl],
                            start=(k == 0),
                            stop=(k == KI - 1),
                        )
                        if after is not None:
                            add_dep_helper(e.ins, after.ins, sync=False)
                        after = e
                    nc.vector.tensor_scalar_add(
                        xp_sb[:, m, sl], ps[:], bcb_sb[:, m : m + 1]
                    )
                    return after

                prev = None
                for m in range(KH):
                    prev = g1_emit(0, m, prev)
                g1_jobs = [(j, m) for j in range(1, ng1) for m in range(KH)]
                g1_i = 0

                # GEMM2 work units: (par, lo, hi, ot, gate) = KH matmuls
                # (N=hi-lo) + 1 DVE drain (+ the piece's output DMA on the
                # last ot). A piece reads h slots that exist after step
                # `gate`. The final parity-0 chunk is split 56+8 so only the
                # 8 columns depending on step 511 trail the loop.
                g2_jobs = []
                for c in range(nch2):
                    for par in range(2):
                        lo, hi = c * CH2, (c + 1) * CH2
                        gate = 16 * c + 16 - par
                        if par == 0 and c == nch2 - 1:
                            for ot in range(KO):
                                g2_jobs.append((0, lo, hi - BL, ot, gate - 2))
                            for ot in range(KO):
                                g2_jobs.append((0, hi - BL, hi, ot, gate))
                        else:
                            for ot in range(KO):
                                g2_jobs.append((par, lo, hi, ot, gate))
                g2_i = 0

                def g2_emit(job, after):
                    par, lo, hi, ot, _ = job
                    base = BL if par == 0 else 0
                    sl = slice(lo, hi)
                    hsl = slice(base + lo, base + hi)
                    ps = g2p.tile([P, hi - lo], f32, tag="g2ps",
                                  padded_shape=[P, CH2])
                    for k in range(KH):
                        rhs = (
                            hX[par][:, k, hsl] if k < 2 else hY[par][:, k - 2, hsl]
                        )
                        e = nc.tensor.matmul(
                            ps[:],
                            wff_sb[:, k, ot * P : (ot + 1) * P],
                            rhs,
                            start=(k == 0),
                            stop=(k == KH - 1),
                        )
                        if after is not None:
                            add_dep_helper(e.ins, after.ins, sync=False)
                        after = e
                    nc.vector.tensor_scalar_add(
                        out_sb[:, ot, par, sl], ps[:], bfb_sb[:, ot : ot + 1]
                    )
                    if ot == KO - 1:
                        eng = nc.gpsimd if (lo // CH2 + par) % 2 and lo // CH2 < 29 else nc.sync
                        eng.dma_start(y_r[:, :, par, sl], out_sb[:, :, par, sl])
                    return after

                # ---- the recurrence ----
                def kmm(zX, zY, m, k, t):
                    rX, rY = hX[t % 2], hY[t % 2]
                    rof = (t // 2) * BL
                    zt = zX[:, m, :] if m < 2 else zY[:, m - 2, :]
                    rhs = (
                        rX[:, k, rof : rof + BL]
                        if k < 2
                        else rY[:, k - 2, rof : rof + BL]
                    )
                    return nc.tensor.matmul(
                        zt,
                        whh_sb[:, k, m * P : (m + 1) * P],
                        rhs,
                        start=False,
                        stop=(m < 2 and k == 1) or (m >= 2 and k == 1),
                    )

                def imm(zt, mlo, t):
                    # single identity-matmul per psum half injects xp_t
                    return nc.tensor.matmul(
                        zt[:],
                        eye_sb[:],
                        xp_sb[:, mlo : mlo + 2, t * BL : (t + 1) * BL],
                        start=True,
                        stop=False,
                    )

                def chain(e):
                    nonlocal prev
                    add_dep_helper(e.ins, prev.ins, sync=False)
                    prev = e

                zX = zXp.tile([P, 2, BL], f32, tag="zx")
                zY = zYp.tile([P, 2, BL], f32, tag="zy")
                chain(imm(zY, 2, 0))
                chain(imm(zX, 0, 0))
                for t in range(s):
                    wX, wY = hX[(t + 1) % 2], hY[(t + 1) % 2]
                    wof = ((t + 1) // 2) * BL

                    # Burst order so the X half completes early enough for
                    # tanhX to PIPELINE behind tanhY on ACT (issue-to-issue
                    # ~179ns) instead of serializing (+273): Y*kY, X*kY,
                    # Y*kX (gating, lands past the now-earlier tanhX gate
                    # with no stall), tanhY, X*kX, tanhX.
                    # Y group (gating): k in Y tiles first, k in X tiles last
                    for m in (2, 3):
                        for k in (3, 2):
                            chain(kmm(zX, zY, m, k, t))
                    # next step's zY bank + xp inject fills the X-wait gap
                    if t + 1 < s:
                        zY2 = zYp.tile([P, 2, BL], f32, tag="zy")
                        chain(imm(zY2, 2, t + 1))
                    for m in (2, 3):
                        for k in (0, 1):
                            chain(kmm(zX, zY, m, k, t))
                    nc.scalar.activation(
                        wY[:, :, wof : wof + BL], zY[:], AF.Tanh
                    )
                    # X group
                    for m in (0, 1):
                        for k in (3, 2, 0, 1):
                            chain(kmm(zX, zY, m, k, t))
                    nc.scalar.activation(
                        wX[:, :, wof : wof + BL], zX[:], AF.Tanh
                    )
                    if t + 1 < s:
                        zX2 = zXp.tile([P, 2, BL], f32, tag="zx")
                        chain(imm(zX2, 0, t + 1))
                        zX, zY = zX2, zY2
                    # shadow jobs per step: GEMM1 matmuls first, then GEMM2
                    if g1_i < len(g1_jobs):
                        j, m = g1_jobs[g1_i]
                        prev = g1_emit(j, m, prev)
                        g1_i += 1
                    else:
                        for _ in range(2 if t >= 496 else 1):
                            if g2_i < len(g2_jobs) and g2_jobs[g2_i][4] <= t:
                                prev = g2_emit(g2_jobs[g2_i], prev)
                                g2_i += 1
                # tail of GEMM2 (last chunks need the final steps)
                while g2_i < len(g2_jobs):
                    prev = g2_emit(g2_jobs[g2_i], prev)
                    g2_i += 1

    return nc


def make_in_maps(x, h0, W_ih, W_hh, b_ih, b_hh, W_ff, b_ff, seq_len=S):
    """Host-side sharding + layout prep: per-core input dicts."""
    bf = ml_dtypes.bfloat16
    x = np.asarray(x, np.float32)
    h0 = np.asarray(h0, np.float32)
    wih = np.ascontiguousarray(np.asarray(W_ih, np.float32).T).astype(bf)   # [I, H]
    whh = np.ascontiguousarray(np.asarray(W_hh, np.float32).T).astype(bf)   # [H, H]
    wff = np.ascontiguousarray(np.asarray(W_ff, np.float32).T).astype(bf)   # [H, O]
    bc = np.asarray(b_ih, np.float32) + np.asarray(b_hh, np.float32)
    bcb = np.ascontiguousarray(bc.reshape(KH, P).T)             # [128, KH]
    bfb = np.ascontiguousarray(np.asarray(b_ff, np.float32).reshape(KO, P).T)
    eye = np.eye(P, dtype=np.float32).astype(bf)

    in_maps = []
    for c in range(NCORES):
        xs = x[c * BL : (c + 1) * BL, :seq_len]                 # [BL, s, I]
        xt = np.ascontiguousarray(xs.transpose(2, 1, 0)).reshape(I, seq_len * BL)
        h0t = np.ascontiguousarray(h0[c * BL : (c + 1) * BL].T)  # [H, BL]
        in_maps.append(
            {
                "xt": xt.astype(bf),
                "h0t": h0t.astype(bf),
                "wih": wih,
                "whh": whh,
                "wff": wff,
                "bcb": bcb,
                "bfb": bfb,
                "eye": eye,
            }
        )
    return in_maps


def assemble_output(results, seq_len=S):
    """Per-core y [KO, 128, 2, (s/2)*BL] -> full [B, s, O]."""
    s = seq_len
    outs = []
    for r in results:
        yc = np.asarray(r["y"]).reshape(O, 2, s // 2, BL)
        full = np.empty((O, s, BL), np.float32)
        full[:, 1::2, :] = yc[:, 0]   # par=0: t = 2q+1
        full[:, 0::2, :] = yc[:, 1]   # par=1: t = 2q
        outs.append(full.transpose(2, 1, 0))
    return np.ascontiguousarray(np.concatenate(outs, axis=0))


def _get_finalized_nc(seq_len=S):
    key = ("nc", seq_len)
    if key not in _builder_cache:
        nc = build_nc(seq_len)
        nc.finalize()
        _builder_cache[key] = nc
    return _builder_cache[key]


def run_on_cores(inputs, seq_len=S, **kwargs):
    from concourse.bass_utils import run_bass_kernel_spmd

    nc = _get_finalized_nc(seq_len)
    in_maps = make_in_maps(**inputs, seq_len=seq_len)
    res = run_bass_kernel_spmd(nc, in_maps, core_ids=list(range(NCORES)), **kwargs)
    return res


def kernel(**inputs) -> np.ndarray:
    res = run_on_cores(inputs)
    return assemble_output(res.results)
